# revision 54
# baseline (speedup 1.0000x reference)
"""DeepHisCoM forward pass on 8 Trainium2 NeuronCores.

Strategy: pathway (expert) parallelism -- 8 of the 64 pathways per core.
All three grouped GEMMs run in fp8e4 DoubleRow mode (2 k-rows per PE
cell per cycle); weights are host-prescaled by 16 to clear the fp8
subnormal floor and un-scaled for free inside the LeakyReLU eviction
(lrelu is positive-homogeneous: lrelu(u/16) = lrelu(u)/16).

PSUM evictions are split between the Scalar engine (native Lrelu) and
the Vector engine (mul+max pair) so the PE never stalls on a single
eviction engine and stays at its max p-state clock.

BatchNorm is per-pathway and therefore core-local.  The tail avoids
materializing pn entirely: with a = gamma*rstd and b = beta - mean*a,
  s_row[b]  = sum_j fcw_j*(a_j p_jb + b_j) = (fcw*a)^T p + sum_j fcw_j b_j
  ssq_j     = sum_b pn^2 = B*(a_j^2 var_j + beta_j^2)
so one tiny matmul over p (bf16, partitions 0-7) plus a handful of
[8,1] vector ops produce the AllReduce payload [s_row(2048), ssq, bias].
The s_row partials are DMAed PSUM->DRAM directly.

Post-AllReduce the final math runs on a [128,16] layout (16 batch
elements per partition) instead of a single partition; 1/||pn|| is
computed with a DVE Newton iteration (seeded by the near-constant
B*P ~ 131072 sum of squares) so the Scalar engine needs only the
sigmoid table, which is preloaded by a dummy op during the AllReduce
wait.
"""

import os
import sys

sys.path.insert(0, "/opt/trn_rl_repo")

from contextlib import ExitStack

import ml_dtypes
import numpy as np

import concourse.bacc as bacc
import concourse.bass as bass
import concourse.tile as tile
from concourse import mybir
from concourse.bass_utils import run_bass_kernel_spmd

P_TOT = 64   # pathways
NV = 512     # features per pathway
WID = 256    # hidden width
COV = 16     # covariates
B = 2048     # batch
EPS = 1e-5
SLOPE = 0.2
NCORES = 8
PPC = P_TOT // NCORES  # pathways per core
KT1 = NV // 128        # k-tiles for GEMM1 (4)
KT2 = WID // 128       # k-tiles for GEMM2 / GEMV (2)
KP1 = KT1 // 2         # k-pairs for GEMM1 DoubleRow (2)
MT = WID // 128        # m-tiles (2)
NCH = B // 512         # batch chunks of 512 (4)
# Host premultiplies W1 by 16 (clears the fp8 subnormal floor); W2/W3 stay
# unscaled, so h1/h2 carry the 16x factor through the fp8 pipeline (lrelu is
# positive-homogeneous) and only the GEMV eviction divides it back out.
WSCALE = 16.0
RSC = 1.0 / WSCALE
SEED_RN = 1.0 / 362.03867  # rsqrt seed: ||pn||^2 ~ B*P_TOT = 131072

FP8 = mybir.dt.float8e4
BF16 = mybir.dt.bfloat16
F32 = mybir.dt.float32
AF = mybir.ActivationFunctionType
ALU = mybir.AluOpType
DR = mybir.MatmulPerfMode.DoubleRow

# Native Lrelu runs on hardware; the CPU interpreter doesn't implement it,
# so sim checks set KERNEL_LRELU=0 to use the max(x, 0.2x) fallback.
USE_NATIVE_LRELU = os.environ.get("KERNEL_LRELU", "1") == "1"
# Vector-engine lrelu in one scalar_tensor_tensor (both sources PSUM);
# set KERNEL_VEC1=0 for the two-op mul+max fallback.
VEC1 = os.environ.get("KERNEL_VEC1", "1") == "1"
# GEMV mode: "dr4" = DoubleRow 4-column zero-padded accumulate,
# "tp" = tile_position column packing (plain fp8, baseline-proven).
GEMV_MODE = os.environ.get("KERNEL_GEMV", "dr4")


def _mm(nc, out, lhsT, rhs, ldw=True, **kw):
    """matmul wrapper: ldw=False reuses the previously loaded stationary."""
    mi = nc.tensor.matmul(out, lhsT, rhs, **kw)
    if not ldw:
        mi.ins.ldweights = False
    return mi


def _evict(nc, sc_pool, ps, dst, use_scalar):
    """dst = lrelu(ps), unscaled; ps is a PSUM [128, 2, 512] group.

    Whole groups alternate between the Scalar engine (native Lrelu, one op)
    and a GpSimd+Vector pair (0.2x to scratch on GpSimd, max on Vector) --
    each op reads PSUM only once, which is all the ISA allows.
    """
    src = ps.rearrange("p a b -> p (a b)")
    pp = src.shape[0]
    if use_scalar:
        if USE_NATIVE_LRELU:
            nc.scalar.activation(dst, src, AF.Lrelu, alpha=SLOPE)
            return
        sc = sc_pool.tile([128, 2 * 512], BF16, tag="scf", name="scf")
        nc.scalar.activation(sc[0:pp, :], src, AF.Copy, scale=SLOPE)
        nc.vector.tensor_tensor(dst, src, sc[0:pp, :], ALU.max)
        return
    sc = sc_pool.tile([128, 2 * 512], BF16, tag="scv", name="scv")
    nc.vector.tensor_scalar_mul(sc[0:pp, :], src, SLOPE)
    nc.vector.tensor_tensor(dst, src, sc[0:pp, :], ALU.max)


def _emit(ctx, tc, xt, w12, w3z, w3tp, xcovw, fcwcb, fcbb, cst, out):
    nc = tc.nc

    xt_pool = ctx.enter_context(tc.tile_pool(name="xt_pool", bufs=2))
    w_pool = ctx.enter_context(tc.tile_pool(name="w_pool", bufs=2))
    h1_pool = ctx.enter_context(tc.tile_pool(name="h1_pool", bufs=2))
    h2_pool = ctx.enter_context(tc.tile_pool(name="h2_pool", bufs=5))
    sc_pool = ctx.enter_context(tc.tile_pool(name="sc_pool", bufs=3))
    one = ctx.enter_context(tc.tile_pool(name="one", bufs=1))
    psg = ctx.enter_context(tc.tile_pool(name="psg", bufs=4, space="PSUM"))
    dram = ctx.enter_context(tc.tile_pool(name="dram", bufs=1, space="DRAM"))

    # ---- pathway 0's data first: it gates the first matmul ----
    xt_sb0 = xt_pool.tile([128, KT1, B], FP8, tag="xt", name="xt_sb")
    nc.sync.dma_start(out=xt_sb0[:, 0:2, :], in_=xt[0, :, 0:2, :])
    w12_sb0 = w_pool.tile([128, KT1 + KT2, 256], FP8, tag="w", name="w12_sb")
    nc.sync.dma_start(out=w12_sb0[:], in_=w12[0])
    nc.sync.dma_start(out=xt_sb0[:, 2:4, :], in_=xt[0, :, 2:4, :])

    # ---- persistents (all small; only needed mid-kernel or later) ----
    if GEMV_MODE == "dr4":
        w3_sb = one.tile([128, KT2, PPC, 4], FP8)
        nc.sync.dma_start(out=w3_sb[:], in_=w3z[:])
        w3tp_sb = None
    else:
        w3tp_sb = one.tile([128, 2, KT2, 128], FP8)
        nc.sync.dma_start(out=w3tp_sb[:], in_=w3tp[:])
        w3_sb = None
    # [j, group, (fcw, gamma, beta, B*g^2, B*b^2)] for pathway g*4+j
    cst_sb = one.tile([4, 2, 5], F32)
    nc.sync.dma_start(out=cst_sb[:], in_=cst[:])
    xcovw_sb = one.tile([128, COV, 16], BF16)
    nc.sync.dma_start(out=xcovw_sb[:], in_=xcovw[:])
    fcwcb_sb = one.tile([128, COV], F32)
    nc.sync.dma_start(out=fcwcb_sb[:], in_=fcwcb[:])
    fcbb_sb = one.tile([128, 1], F32)
    nc.sync.dma_start(out=fcbb_sb[:], in_=fcbb[:])
    ones_bf = one.tile([1, 128], BF16)
    nc.vector.memset(ones_bf[:], 1.0)

    # per-group BN state: groups of 4 pathways live on partitions 0-3 with
    # the group index in the free dim, so group 0's whole chain runs
    # mid-kernel while pathways 4-7 are still in their GEMMs.
    p_bf = one.tile([4, 2, B], BF16)
    stats = one.tile([4, 2, NCH, 6], F32)
    mv = one.tile([4, 2, 2], F32)
    ve = one.tile([4, 2], F32)
    rve = one.tile([4, 2], F32)
    rstd = one.tile([4, 2], F32)
    a_sc = one.tile([4, 2], F32)
    fcwa_f = one.tile([4, 2], F32)
    fcwa_bf = one.tile([4, 2], BF16)
    mb = one.tile([4, 2], F32)
    bvec = one.tile([4, 2], F32)
    vr = one.tile([4, 2], F32)
    sfb = one.tile([4, 2, 2], F32)        # [., group, (ssq_j | fcw_j*b_j)]
    ones4 = one.tile([4, 1], F32)
    s_g0 = one.tile([1, B], F32)
    s_row = one.tile([1, B], BF16)
    ssfb = one.tile([1, 2], BF16)         # [ssq partial, fcw*b partial]
    cov_row = one.tile([128, 16], F32)    # covariate term + fc_b, b = p*16+j
    s128 = one.tile([128, 16], BF16)
    tt2 = one.tile([1, 2], BF16)          # [ssq total, bias total]
    nt = one.tile([1, 4], F32)            # Newton scratch: y, t, rb, dummy
    rnrb_bf = one.tile([1, 2], BF16)
    rnb = one.tile([128, 2], F32)
    u128 = one.tile([128, 16], F32)
    out_t = one.tile([128, 16], F32)

    nc.vector.memset(ones4[:], 1.0)

    # ---- covariate term on the Vector engine (head is DMA-bound) ----
    # cov_row = sum_c fcwc_c * xcovw[:, c, :] + fc_b
    nc.vector.tensor_scalar(cov_row[:], xcovw_sb[:, 0, :],
                            fcwcb_sb[:, 0:1], fcbb_sb[:],
                            ALU.mult, ALU.add)
    for c in range(1, COV):
        nc.vector.scalar_tensor_tensor(cov_row[:], xcovw_sb[:, c, :],
                                       fcwcb_sb[:, c:c + 1], cov_row[:],
                                       ALU.mult, ALU.add)

    def _bn_group(g):
        """BN stats + analytic coefficients + s-partial matmuls for one
        group of 4 pathways.  Group 0 runs mid-kernel (hidden under the
        pathways 4-7 GEMMs); only group 1 is on the tail critical path."""
        gs = slice(g, g + 1)
        for s in range(NCH):
            nc.vector.bn_stats(out=stats[:, g, s, :],
                               in_=p_bf[:, g, s * 512:(s + 1) * 512])
        nc.vector.bn_aggr(out=mv[:, g, :], in_=stats[:, g])
        # p is carried at 16x scale, so eps scales by 16^2 (exact algebra)
        nc.vector.tensor_scalar_add(ve[:, gs], mv[:, g, 1:2],
                                    EPS * WSCALE * WSCALE)
        nc.vector.reciprocal(rve[:, gs], ve[:, gs])
        nc.scalar.activation(rstd[:, gs], rve[:, gs], AF.Sqrt)
        nc.vector.tensor_tensor(a_sc[:, gs], cst_sb[:, g, 1:2], rstd[:, gs],
                                ALU.mult)
        nc.vector.tensor_tensor(fcwa_f[:, gs], cst_sb[:, g, 0:1],
                                a_sc[:, gs], ALU.mult)
        nc.scalar.activation(fcwa_bf[:, gs], fcwa_f[:, gs], AF.Copy)
        # b = beta - mean*a; sfb[., 1] = fcw*b
        nc.vector.tensor_tensor(mb[:, gs], mv[:, g, 0:1], a_sc[:, gs],
                                ALU.mult)
        nc.vector.tensor_tensor(bvec[:, gs], cst_sb[:, g, 2:3], mb[:, gs],
                                ALU.subtract)
        nc.vector.tensor_tensor(sfb[:, g, 1:2], cst_sb[:, g, 0:1],
                                bvec[:, gs], ALU.mult)
        # sfb[., 0] = ssq_j = B*gamma^2 * var/(var+eps) + B*beta^2
        nc.vector.tensor_tensor(vr[:, gs], mv[:, g, 1:2], rve[:, gs],
                                ALU.mult)
        nc.vector.scalar_tensor_tensor(sfb[:, g, 0:1], vr[:, gs],
                                       cst_sb[:, g, 3:4], cst_sb[:, g, 4:5],
                                       ALU.mult, ALU.add)
        # s partials over this group's 4 pathways
        for sh in range(2):
            sp = psg.tile([128, 2, 512], F32, tag="g", name="sp")
            for n in range(2):
                _mm(nc, sp[0:1, n, :], fcwa_bf[:, gs],
                    p_bf[:, g, (2 * sh + n) * 512:(2 * sh + n + 1) * 512],
                    ldw=(n == 0), start=True, stop=True)
            srcs = sp[0:1, :, :].rearrange("p a b -> p (a b)")
            if g == 0:
                dsts = s_g0[:, sh * 1024:(sh + 1) * 1024]
                if sh == 0:
                    nc.scalar.activation(dsts, srcs, AF.Copy)
                else:
                    nc.vector.tensor_scalar_mul(dsts, srcs, 1.0)
            else:
                # fold in group 0's partial during the eviction
                nc.vector.scalar_tensor_tensor(
                    s_row[:, sh * 1024:(sh + 1) * 1024], srcs, 1.0,
                    s_g0[:, sh * 1024:(sh + 1) * 1024], ALU.mult, ALU.add)

    # ---- pathway loop ----
    h2_tiles = []
    for p in range(PPC):
        if p == 0:
            xt_sb, w12_sb = xt_sb0, w12_sb0
        else:
            xt_sb = xt_pool.tile([128, KT1, B], FP8, tag="xt", name="xt_sb")
            nc.sync.dma_start(out=xt_sb[:, 0:2, :], in_=xt[p, :, 0:2, :])
            nc.sync.dma_start(out=xt_sb[:, 2:4, :], in_=xt[p, :, 2:4, :])
            w12_sb = w_pool.tile([128, KT1 + KT2, 256], FP8, tag="w",
                                 name="w12_sb")
            nc.sync.dma_start(out=w12_sb[:], in_=w12[p])

        h1_sb = h1_pool.tile([128, MT, B], FP8, tag="h1", name="h1_sb")
        h2_sb = h2_pool.tile([128, KT2, B], FP8, tag="h2", name="h2_sb")

        # GEMM1: psum[o, b] = sum_i (16*W1)[i, o] * xT[i, b], fp8 DoubleRow.
        # Half-batch PSUM groups (2 banks) keep 4 groups in flight; whole
        # groups alternate between the Scalar and Vector eviction paths.
        # scalar Lrelu is 1 op, the vector mul+max pair is ~2x the time, so
        # scalar takes 5 of each pathway's 8 GEMM evict-groups; no two
        # vector groups are adjacent (vector bursts stall the PE on PSUM).
        sched = (1, 0, 1, 1, 0, 1, 1, 0)
        evi = [0]
        for m in range(MT):
            ph = [psg.tile([128, 2, 512], F32, tag="g", name="ps")
                  for _ in range(2)]
            for kp in range(KP1):
                for h in range(2):
                    for n in range(2):
                        _mm(nc, ph[h][:, n],
                            w12_sb[:, 2 * kp:2 * kp + 2,
                                   m * 128:(m + 1) * 128],
                            xt_sb[:, 2 * kp:2 * kp + 2,
                                  (2 * h + n) * 512:(2 * h + n + 1) * 512],
                            ldw=(h == 0 and n == 0),
                            start=(kp == 0),
                            stop=(kp == KP1 - 1),
                            perf_mode=DR)
            for h in range(2):
                _evict(nc, sc_pool, ph[h],
                       h1_sb[:, m, 2 * h * 512:(2 * h + 2) * 512],
                       use_scalar=sched[evi[0]])
                evi[0] += 1

        # GEMM2: one k-pair (K=256); the stationary is shared by all 4 chunks
        for m in range(MT):
            ph = [psg.tile([128, 2, 512], F32, tag="g", name="ps")
                  for _ in range(2)]
            for h in range(2):
                for n in range(2):
                    _mm(nc, ph[h][:, n],
                        w12_sb[:, KT1:KT1 + 2, m * 128:(m + 1) * 128],
                        h1_sb[:, 0:2,
                              (2 * h + n) * 512:(2 * h + n + 1) * 512],
                        ldw=(h == 0 and n == 0),
                        start=True,
                        stop=True,
                        perf_mode=DR)
            for h in range(2):
                _evict(nc, sc_pool, ph[h],
                       h2_sb[:, m, 2 * h * 512:(2 * h + 2) * 512],
                       use_scalar=sched[evi[0]])
                evi[0] += 1
        h2_tiles.append(h2_sb)

        # GEMV for a group of 4 pathways.  p is kept at 16x scale (p16 =
        # lrelu(16*h2w3)); BatchNorm absorbs the factor exactly once EPS is
        # scaled by 16^2, so no eviction rescale is needed anywhere.
        if p % 4 == 3 and GEMV_MODE == "dr4":
            # DoubleRow, zero-padded 4-column stationaries: pathway j's
            # weights live in column j, other columns are zero, and the 4
            # matmuls accumulate into the same [4, 512] PSUM chunk (+0 rows).
            # Both chunk-pair PSUM tiles stay live so each pathway's
            # stationary is loaded once.
            g = p // 4
            vt = sc_pool.tile([4, B], BF16, tag="vt", name="vt")
            pvs = [psg.tile([128, 2, 512], F32, tag="g", name="pv")
                   for _ in range(2)]
            for j in range(4):
                for nh in range(2):
                    for n in range(2):
                        _mm(nc, pvs[nh][0:4, n, :],
                            w3_sb[:, :, g * 4 + j, :],
                            h2_tiles[g * 4 + j][
                                :, :, (2 * nh + n) * 512:
                                (2 * nh + n + 1) * 512],
                            ldw=(nh == 0 and n == 0),
                            start=(j == 0),
                            stop=(j == 3),
                            perf_mode=DR)
            for nh in range(2):
                dstv = vt[:, nh * 1024:(nh + 1) * 1024]
                _evict(nc, sc_pool, pvs[nh][0:4, :, :], dstv,
                       use_scalar=(nh == 0))
                nc.sync.dma_start(
                    out=p_bf[:, g, nh * 1024:(nh + 1) * 1024],
                    in_=dstv)
            _bn_group(g)
        elif p % 4 == 3:
            # tile_position column packing, plain fp8 (baseline-proven).
            g = p // 4
            vt = sc_pool.tile([128, 4, 512], BF16, tag="vt", name="vt")
            for nh in range(2):
                pv = psg.tile([128, 2, 512], F32, tag="g", name="pv")
                for j in range(4):
                    for k in range(KT2):
                        for n in range(2):
                            _mm(nc, pv[32 * j:32 * j + 32, n, :],
                                w3tp_sb[:, g, k, 32 * j:32 * j + 32],
                                h2_tiles[g * 4 + j][
                                    :, k, (2 * nh + n) * 512:
                                    (2 * nh + n + 1) * 512],
                                ldw=(n == 0),
                                start=(k == 0),
                                stop=(k == KT2 - 1),
                                tile_position=(0, 32 * j))
                dstv = vt.rearrange("p a b -> p (a b)")[
                    :, nh * 1024:(nh + 1) * 1024]
                _evict(nc, sc_pool, pv, dstv, use_scalar=(nh == 0))
                nc.sync.dma_start(
                    out=p_bf[:, g, nh * 1024:(nh + 1) * 1024],
                    in_=vt[0:97:32, :, :].rearrange("p a b -> p (a b)")[
                        :, nh * 1024:(nh + 1) * 1024])
            _bn_group(g)

    # cross-partition reduce of [ssq_j, fcw_j*b_j] via a tiny fp32 matmul
    red = psg.tile([128, 2, 512], F32, tag="g", name="red")
    for g in range(2):
        nc.tensor.matmul(red[0:1, 0, 0:2], ones4[:], sfb[:, g, :],
                         start=(g == 0), stop=(g == 1))
    nc.scalar.activation(ssfb[:], red[0:1, 0, 0:2], AF.Copy)

    # one AllReduce: [s_row(2048), ssq, bias] in bf16
    ar_in = dram.tile([1, B + 2], BF16)
    ar_out = dram.tile([1, B + 2], BF16)
    nc.sync.dma_start(out=ar_in[0:1, 0:B], in_=s_row[:])
    nc.sync.dma_start(out=ar_in[0:1, B:B + 2], in_=ssfb[:])
    # dummy sigmoid depends on the last s_row eviction: it runs after every
    # other scalar op, loading the sigmoid table during the AllReduce wait.
    nc.scalar.activation(nt[:, 3:4], s_row[0:1, B - 1:B], AF.Sigmoid)
    nc.gpsimd.collective_compute(
        "AllReduce",
        ALU.add,
        replica_groups=[list(range(NCORES))],
        ins=[ar_in.opt()],
        outs=[ar_out.opt()],
    )
    nc.sync.dma_start(out=s128[:],
                      in_=ar_out[0:1, 0:B].rearrange("one (p j) -> p (one j)",
                                                     p=128))
    nc.sync.dma_start(out=tt2[:], in_=ar_out[0:1, B:B + 2])

    # rn = rsqrt(ssq_tot) via DVE: seed from reciprocal, 2 Newton steps
    y = nt[:, 0:1]
    t = nt[:, 1:2]
    rb = nt[:, 2:3]
    nc.vector.reciprocal(y[:], tt2[:, 0:1])
    nc.vector.tensor_scalar_mul(y[:], y[:], 1.0 / SEED_RN)  # y0 = S/c
    for _ in range(1):
        nc.vector.tensor_tensor(t[:], y[:], y[:], ALU.mult)
        nc.vector.tensor_tensor(t[:], t[:], tt2[:, 0:1], ALU.mult)
        nc.vector.tensor_scalar(t[:], t[:], -0.5, 1.5, ALU.mult, ALU.add)
        nc.vector.tensor_tensor(y[:], y[:], t[:], ALU.mult)
    nc.vector.tensor_tensor(rb[:], y[:], tt2[:, 1:2], ALU.mult)  # rn*bias
    nc.vector.tensor_scalar_mul(rnrb_bf[:, 0:1], y[:], 1.0)
    nc.vector.tensor_scalar_mul(rnrb_bf[:, 1:2], rb[:], 1.0)
    # broadcast [rn, rn*bias] to all 128 partitions via the PE
    bc = psg.tile([128, 2, 512], F32, tag="g", name="bc")
    nc.tensor.matmul(bc[0:128, 0, 0:2], ones_bf[:], rnrb_bf[:],
                     start=True, stop=True)
    nc.scalar.activation(rnb[:], bc[:, 0, 0:2], AF.Copy)
    # out = sigmoid(s*rn + cov + fc_b + rn*bias)
    nc.vector.scalar_tensor_tensor(u128[:], s128[:], rnb[:, 0:1], cov_row[:],
                                   ALU.mult, ALU.add)
    nc.scalar.activation(out_t[:], u128[:], AF.Sigmoid, bias=rnb[:, 1:2])
    nc.sync.dma_start(out=out.rearrange("(p j) one -> p (j one)", p=128),
                      in_=out_t[:])


def _dedup_ldweights(nc):
    """Drop InstLdweights that reload the stationary already in the PE.

    The Tile lowering splits every matmul into Ldweights + Matmult; for the
    n-inner loops above, consecutive groups load the identical stationary 4x.
    Only wait-free exact duplicates (same AP/offset/dtype/perf_mode/tile) with
    no intervening PE weight change are removed, so semaphore deps survive on
    the group's first load.
    """
    removed = 0
    for fn in nc.m.functions:
        for b in fn.blocks:
            last_sig = None
            keep = []
            for i in b.instructions:
                cn = type(i).__name__
                if cn == "InstLdweights":
                    ap = i.ins[0]
                    sig = (str(ap.ap), ap.offset, str(ap.dtype),
                           getattr(ap, "memref", None), str(i.perf_mode),
                           str(i.tile_position), str(i.tile_size),
                           str(i.is_transpose))
                    has_wait = bool(i.sync_info and i.sync_info.on_wait)
                    if sig == last_sig and not has_wait:
                        removed += 1
                        continue
                    last_sig = sig
                keep.append(i)
            b.instructions[:] = keep
    return removed


_NC = None


def _get_compiled():
    global _NC
    if _NC is None:
        nc = bacc.Bacc("TRN2", target_bir_lowering=False, debug=False,
                       num_devices=NCORES)
        xt = nc.dram_tensor("xt", [PPC, 128, KT1, B], FP8,
                            kind="ExternalInput").ap()
        w12 = nc.dram_tensor("w12", [PPC, 128, KT1 + KT2, 256], FP8,
                             kind="ExternalInput").ap()
        w3z = nc.dram_tensor("w3z", [128, KT2, PPC, 4], FP8,
                             kind="ExternalInput").ap()
        w3tp = nc.dram_tensor("w3tp", [128, 2, KT2, 128], FP8,
                              kind="ExternalInput").ap()
        xcovw = nc.dram_tensor("xcovw", [128, COV, 16], BF16,
                               kind="ExternalInput").ap()
        fcwcb = nc.dram_tensor("fcwcb", [128, COV], F32,
                               kind="ExternalInput").ap()
        fcbb = nc.dram_tensor("fcbb", [128, 1], F32,
                              kind="ExternalInput").ap()
        cst = nc.dram_tensor("cst", [4, 2, 5], F32,
                             kind="ExternalInput").ap()
        out = nc.dram_tensor("out", [B, 1], F32, kind="ExternalOutput").ap()
        with tile.TileContext(nc) as tc:
            with ExitStack() as ctx:
                _emit(ctx, tc, xt, w12, w3z, w3tp, xcovw, fcwcb, fcbb, cst,
                      out)
        _dedup_ldweights(nc)
        nc.compile()
        _NC = nc
    return _NC


def _shard(inputs):
    x = np.asarray(inputs["x"], np.float32)
    W1 = np.asarray(inputs["W1"], np.float32)
    W2 = np.asarray(inputs["W2"], np.float32)
    W3 = np.asarray(inputs["W3"], np.float32)
    gamma = np.asarray(inputs["gamma"], np.float32)
    beta = np.asarray(inputs["beta"], np.float32)
    fc_w = np.asarray(inputs["fc_w"], np.float32)
    fc_b = np.asarray(inputs["fc_b"], np.float32)

    fp8 = ml_dtypes.float8_e4m3
    xm = x[:, :P_TOT * NV].reshape(B, P_TOT, NV)
    # covariates laid out [partition(=b//16), c, j(=b%16)] for the DVE pass
    xcov = x[:, P_TOT * NV:P_TOT * NV + COV]          # [B, COV]
    xcovw = np.ascontiguousarray(
        xcov.reshape(128, 16, COV).transpose(0, 2, 1)).astype(
            ml_dtypes.bfloat16)
    fcwcb = np.broadcast_to(fc_w[P_TOT:P_TOT + COV].reshape(1, COV),
                            (128, COV)).astype(np.float32).copy()
    fcbb = np.full((128, 1), float(fc_b[0]), np.float32)

    maps = []
    for c in range(NCORES):
        sl = slice(c * PPC, (c + 1) * PPC)
        # xt: [PPC, 128, KT1, B]; feature f = kt*128 + kp
        xt_c = np.ascontiguousarray(
            xm[:, sl, :].transpose(1, 2, 0)            # [PPC, NV, B]
            .reshape(PPC, KT1, 128, B).transpose(0, 2, 1, 3)).astype(fp8)
        # w12: [PPC, 128, 6, 256] = [W1 k-tiles | W2 k-tiles]; only W1 is
        # prescaled -- the 16x rides through h1/h2/p and BN absorbs it.
        w1_c = (W1[sl] * WSCALE).reshape(PPC, KT1, 128, WID).transpose(
            0, 2, 1, 3)
        w2_c = W2[sl].reshape(PPC, KT2, 128, WID).transpose(
            0, 2, 1, 3)
        w12_c = np.concatenate([w1_c, w2_c], axis=2)
        w12_c = np.ascontiguousarray(w12_c).astype(fp8)
        # w3z: [128, KT2, PPC, 4]; pathway q's weights in column q%4
        w3z_c = np.zeros((128, KT2, PPC, 4), np.float32)
        # w3tp: [128, 2, KT2, 128]; pathway g*4+j in column 32*j
        w3tp_c = np.zeros((128, 2, KT2, 128), np.float32)
        for g in range(2):
            for j in range(4):
                wj = W3[c * PPC + g * 4 + j].reshape(KT2, 128)
                w3z_c[:, :, g * 4 + j, j] = wj.T
                w3tp_c[:, g, :, 32 * j] = wj.T
        w3z_c = w3z_c.astype(fp8)
        w3tp_c = w3tp_c.astype(fp8)
        gam = gamma[sl].astype(np.float32)
        bet = beta[sl].astype(np.float32)
        cst_c = np.stack([
            fc_w[sl, 0].astype(np.float32),
            gam,
            bet,
            B * gam * gam,
            B * bet * bet,
        ], axis=1).astype(np.float32)
        # [pathway, 5] -> [j, group, 5] with pathway = group*4 + j
        cst_c = np.ascontiguousarray(
            cst_c.reshape(2, 4, 5).transpose(1, 0, 2))
        maps.append({
            "xt": xt_c,
            "w12": w12_c,
            "w3z": w3z_c,
            "w3tp": w3tp_c,
            "xcovw": xcovw,
            "fcwcb": fcwcb,
            "fcbb": fcbb,
            "cst": np.ascontiguousarray(cst_c),
        })
    return maps


def kernel(**inputs) -> np.ndarray:
    nc = _get_compiled()
    maps = _shard(inputs)
    res = run_bass_kernel_spmd(nc, maps, list(range(NCORES)))
    return np.asarray(res.results[0]["out"], np.float32)


def kernel_traced(**inputs):
    """Like kernel() but with NTFF profiling; returns (out, BassKernelResults)."""
    nc = _get_compiled()
    maps = _shard(inputs)
    res = run_bass_kernel_spmd(nc, maps, list(range(NCORES)), trace=True)
    return np.asarray(res.results[0]["out"], np.float32), res


# revision 55
# speedup vs baseline: 1.0129x; 1.0129x over previous
"""DeepHisCoM forward pass on 8 Trainium2 NeuronCores.

Strategy: pathway (expert) parallelism -- 8 of the 64 pathways per core.
All three grouped GEMMs run in fp8e4 DoubleRow mode (2 k-rows per PE
cell per cycle); weights are host-prescaled by 16 to clear the fp8
subnormal floor and un-scaled for free inside the LeakyReLU eviction
(lrelu is positive-homogeneous: lrelu(u/16) = lrelu(u)/16).

PSUM evictions are split between the Scalar engine (native Lrelu) and
the Vector engine (mul+max pair) so the PE never stalls on a single
eviction engine and stays at its max p-state clock.

BatchNorm is per-pathway and therefore core-local.  The tail avoids
materializing pn entirely: with a = gamma*rstd and b = beta - mean*a,
  s_row[b]  = sum_j fcw_j*(a_j p_jb + b_j) = (fcw*a)^T p + sum_j fcw_j b_j
  ssq_j     = sum_b pn^2 = B*(a_j^2 var_j + beta_j^2)
so one tiny matmul over p (bf16, partitions 0-7) plus a handful of
[8,1] vector ops produce the AllReduce payload [s_row(2048), ssq, bias].
The s_row partials are DMAed PSUM->DRAM directly.

Post-AllReduce the final math runs on a [128,16] layout (16 batch
elements per partition) instead of a single partition; 1/||pn|| is
computed with a DVE Newton iteration (seeded by the near-constant
B*P ~ 131072 sum of squares) so the Scalar engine needs only the
sigmoid table, which is preloaded by a dummy op during the AllReduce
wait.
"""

import os
import sys

sys.path.insert(0, "/opt/trn_rl_repo")

from contextlib import ExitStack

import ml_dtypes
import numpy as np

import concourse.bacc as bacc
import concourse.bass as bass
import concourse.tile as tile
from concourse import mybir
from concourse.bass_utils import run_bass_kernel_spmd

P_TOT = 64   # pathways
NV = 512     # features per pathway
WID = 256    # hidden width
COV = 16     # covariates
B = 2048     # batch
EPS = 1e-5
SLOPE = 0.2
NCORES = 8
PPC = P_TOT // NCORES  # pathways per core
KT1 = NV // 128        # k-tiles for GEMM1 (4)
KT2 = WID // 128       # k-tiles for GEMM2 / GEMV (2)
KP1 = KT1 // 2         # k-pairs for GEMM1 DoubleRow (2)
MT = WID // 128        # m-tiles (2)
NCH = B // 512         # batch chunks of 512 (4)
# Host premultiplies W1 by 16 (clears the fp8 subnormal floor); W2/W3 stay
# unscaled, so h1/h2 carry the 16x factor through the fp8 pipeline (lrelu is
# positive-homogeneous) and only the GEMV eviction divides it back out.
WSCALE = 16.0
RSC = 1.0 / WSCALE
SEED_RN = 1.0 / 362.03867  # rsqrt seed: ||pn||^2 ~ B*P_TOT = 131072

FP8 = mybir.dt.float8e4
BF16 = mybir.dt.bfloat16
F32 = mybir.dt.float32
AF = mybir.ActivationFunctionType
ALU = mybir.AluOpType
DR = mybir.MatmulPerfMode.DoubleRow

# Native Lrelu runs on hardware; the CPU interpreter doesn't implement it,
# so sim checks set KERNEL_LRELU=0 to use the max(x, 0.2x) fallback.
USE_NATIVE_LRELU = os.environ.get("KERNEL_LRELU", "1") == "1"
# Vector-engine lrelu in one scalar_tensor_tensor (both sources PSUM);
# set KERNEL_VEC1=0 for the two-op mul+max fallback.
VEC1 = os.environ.get("KERNEL_VEC1", "1") == "1"
# GEMV mode: "dr4" = DoubleRow 4-column zero-padded accumulate,
# "tp" = tile_position column packing (plain fp8, baseline-proven).
GEMV_MODE = os.environ.get("KERNEL_GEMV", "dr4")


def _mm(nc, out, lhsT, rhs, ldw=True, **kw):
    """matmul wrapper: ldw=False reuses the previously loaded stationary."""
    mi = nc.tensor.matmul(out, lhsT, rhs, **kw)
    if not ldw:
        mi.ins.ldweights = False
    return mi


def _evict(nc, sc_pool, ps, dst, use_scalar):
    """dst = lrelu(ps), unscaled; ps is a PSUM [128, 2, 512] group.

    Whole groups alternate between the Scalar engine (native Lrelu, one op)
    and a GpSimd+Vector pair (0.2x to scratch on GpSimd, max on Vector) --
    each op reads PSUM only once, which is all the ISA allows.
    """
    src = ps.rearrange("p a b -> p (a b)")
    pp = src.shape[0]
    if use_scalar:
        if USE_NATIVE_LRELU:
            nc.scalar.activation(dst, src, AF.Lrelu, alpha=SLOPE)
            return
        sc = sc_pool.tile([128, 2 * 512], BF16, tag="scf", name="scf")
        nc.scalar.activation(sc[0:pp, :], src, AF.Copy, scale=SLOPE)
        nc.vector.tensor_tensor(dst, src, sc[0:pp, :], ALU.max)
        return
    sc = sc_pool.tile([128, 2 * 512], BF16, tag="scv", name="scv")
    nc.vector.tensor_scalar_mul(sc[0:pp, :], src, SLOPE)
    nc.vector.tensor_tensor(dst, src, sc[0:pp, :], ALU.max)


def _emit(ctx, tc, xt, w12, w3z, w3tp, xcovw, fcwcb, fcbb, cst, out):
    nc = tc.nc

    xt_pool = ctx.enter_context(tc.tile_pool(name="xt_pool", bufs=2))
    w_pool = ctx.enter_context(tc.tile_pool(name="w_pool", bufs=2))
    h1_pool = ctx.enter_context(tc.tile_pool(name="h1_pool", bufs=2))
    h2_pool = ctx.enter_context(tc.tile_pool(name="h2_pool", bufs=5))
    sc_pool = ctx.enter_context(tc.tile_pool(name="sc_pool", bufs=3))
    one = ctx.enter_context(tc.tile_pool(name="one", bufs=1))
    psg = ctx.enter_context(tc.tile_pool(name="psg", bufs=3, space="PSUM"))
    pss = ctx.enter_context(tc.tile_pool(name="pss", bufs=1, space="PSUM"))
    dram = ctx.enter_context(tc.tile_pool(name="dram", bufs=1, space="DRAM"))

    # ---- pathway 0's data first: it gates the first matmul ----
    xt_sb0 = xt_pool.tile([128, KT1, B], FP8, tag="xt", name="xt_sb")
    nc.sync.dma_start(out=xt_sb0[:, 0:2, :], in_=xt[0, :, 0:2, :])
    w12_sb0 = w_pool.tile([128, KT1 + KT2, 256], FP8, tag="w", name="w12_sb")
    nc.sync.dma_start(out=w12_sb0[:], in_=w12[0])
    nc.sync.dma_start(out=xt_sb0[:, 2:4, :], in_=xt[0, :, 2:4, :])

    # ---- persistents (all small; only needed mid-kernel or later) ----
    if GEMV_MODE == "dr4":
        w3_sb = one.tile([128, KT2, PPC, 4], FP8)
        nc.sync.dma_start(out=w3_sb[:], in_=w3z[:])
        w3tp_sb = None
    else:
        w3tp_sb = one.tile([128, 2, KT2, 128], FP8)
        nc.sync.dma_start(out=w3tp_sb[:], in_=w3tp[:])
        w3_sb = None
    # [j, group, (fcw, gamma, beta, B*g^2, B*b^2)] for pathway g*4+j
    cst_sb = one.tile([4, 2, 5], F32)
    nc.sync.dma_start(out=cst_sb[:], in_=cst[:])
    xcovw_sb = one.tile([128, COV, 16], BF16)
    nc.sync.dma_start(out=xcovw_sb[:], in_=xcovw[:])
    fcwcb_sb = one.tile([128, COV], F32)
    nc.sync.dma_start(out=fcwcb_sb[:], in_=fcwcb[:])
    fcbb_sb = one.tile([128, 1], F32)
    nc.sync.dma_start(out=fcbb_sb[:], in_=fcbb[:])
    ones_bf = one.tile([1, 128], BF16)
    nc.vector.memset(ones_bf[:], 1.0)

    # per-group BN state: groups of 4 pathways live on partitions 0-3 with
    # the group index in the free dim, so group 0's whole chain runs
    # mid-kernel while pathways 4-7 are still in their GEMMs.
    p_bf = one.tile([4, 2, B], BF16)
    stats = one.tile([4, 2, NCH, 6], F32)
    mv = one.tile([4, 2, 2], F32)
    ve = one.tile([4, 2], F32)
    rve = one.tile([4, 2], F32)
    rstd = one.tile([4, 2], F32)
    a_sc = one.tile([4, 2], F32)
    fcwa_f = one.tile([4, 2], F32)
    fcwa_bf = one.tile([4, 2], BF16)
    mb = one.tile([4, 2], F32)
    bvec = one.tile([4, 2], F32)
    vr = one.tile([4, 2], F32)
    sfb = one.tile([4, 2, 2], F32)        # [., group, (ssq_j | fcw_j*b_j)]
    ones4 = one.tile([4, 1], F32)
    s_g0 = one.tile([1, B], F32)
    s_row = one.tile([1, B], F32)
    ssfb = one.tile([1, 2], F32)         # [ssq partial, fcw*b partial]
    cov_row = one.tile([128, 16], F32)    # covariate term + fc_b, b = p*16+j
    s128 = one.tile([128, 16], F32)
    tt2 = one.tile([1, 2], F32)          # [ssq total, bias total]
    nt = one.tile([1, 4], F32)            # Newton scratch: y, t, rb, dummy
    rnrb_bf = one.tile([1, 2], BF16)
    rnb = one.tile([128, 2], F32)
    u128 = one.tile([128, 16], F32)
    out_t = one.tile([128, 16], F32)

    nc.vector.memset(ones4[:], 1.0)

    # ---- covariate term on the Vector engine (head is DMA-bound) ----
    # cov_row = sum_c fcwc_c * xcovw[:, c, :] + fc_b
    nc.vector.tensor_scalar(cov_row[:], xcovw_sb[:, 0, :],
                            fcwcb_sb[:, 0:1], fcbb_sb[:],
                            ALU.mult, ALU.add)
    for c in range(1, COV):
        nc.vector.scalar_tensor_tensor(cov_row[:], xcovw_sb[:, c, :],
                                       fcwcb_sb[:, c:c + 1], cov_row[:],
                                       ALU.mult, ALU.add)

    def _bn_group(g):
        """BN stats + analytic coefficients + s-partial matmuls for one
        group of 4 pathways.  Group 0 runs mid-kernel (hidden under the
        pathways 4-7 GEMMs); only group 1 is on the tail critical path."""
        gs = slice(g, g + 1)
        for s in range(NCH):
            nc.vector.bn_stats(out=stats[:, g, s, :],
                               in_=p_bf[:, g, s * 512:(s + 1) * 512])
        nc.vector.bn_aggr(out=mv[:, g, :], in_=stats[:, g])
        # p is carried at 16x scale, so eps scales by 16^2 (exact algebra)
        nc.vector.tensor_scalar_add(ve[:, gs], mv[:, g, 1:2],
                                    EPS * WSCALE * WSCALE)
        nc.vector.reciprocal(rve[:, gs], ve[:, gs])
        nc.scalar.activation(rstd[:, gs], rve[:, gs], AF.Sqrt)
        nc.vector.tensor_tensor(a_sc[:, gs], cst_sb[:, g, 1:2], rstd[:, gs],
                                ALU.mult)
        nc.vector.tensor_tensor(fcwa_f[:, gs], cst_sb[:, g, 0:1],
                                a_sc[:, gs], ALU.mult)
        nc.scalar.activation(fcwa_bf[:, gs], fcwa_f[:, gs], AF.Copy)
        # b = beta - mean*a; sfb[., 1] = fcw*b
        nc.vector.tensor_tensor(mb[:, gs], mv[:, g, 0:1], a_sc[:, gs],
                                ALU.mult)
        nc.vector.tensor_tensor(bvec[:, gs], cst_sb[:, g, 2:3], mb[:, gs],
                                ALU.subtract)
        nc.vector.tensor_tensor(sfb[:, g, 1:2], cst_sb[:, g, 0:1],
                                bvec[:, gs], ALU.mult)
        # sfb[., 0] = ssq_j = B*gamma^2 * var/(var+eps) + B*beta^2
        nc.vector.tensor_tensor(vr[:, gs], mv[:, g, 1:2], rve[:, gs],
                                ALU.mult)
        nc.vector.scalar_tensor_tensor(sfb[:, g, 0:1], vr[:, gs],
                                       cst_sb[:, g, 3:4], cst_sb[:, g, 4:5],
                                       ALU.mult, ALU.add)
        # s partials over this group's 4 pathways
        for sh in range(2):
            sp = pss.tile([128, 2, 512], F32, tag="s", name="sp")
            for n in range(2):
                _mm(nc, sp[0:1, n, :], fcwa_bf[:, gs],
                    p_bf[:, g, (2 * sh + n) * 512:(2 * sh + n + 1) * 512],
                    ldw=(n == 0), start=True, stop=True)
            srcs = sp[0:1, :, :].rearrange("p a b -> p (a b)")
            if g == 0:
                dsts = s_g0[:, sh * 1024:(sh + 1) * 1024]
                if sh == 0:
                    nc.scalar.activation(dsts, srcs, AF.Copy)
                else:
                    nc.vector.tensor_scalar_mul(dsts, srcs, 1.0)
            else:
                # fold in group 0's partial during the eviction
                nc.vector.scalar_tensor_tensor(
                    s_row[:, sh * 1024:(sh + 1) * 1024], srcs, 1.0,
                    s_g0[:, sh * 1024:(sh + 1) * 1024], ALU.mult, ALU.add)

    # ---- pathway loop ----
    h2_tiles = []
    for p in range(PPC):
        if p == 0:
            xt_sb, w12_sb = xt_sb0, w12_sb0
        else:
            xt_sb = xt_pool.tile([128, KT1, B], FP8, tag="xt", name="xt_sb")
            nc.sync.dma_start(out=xt_sb[:, 0:2, :], in_=xt[p, :, 0:2, :])
            nc.sync.dma_start(out=xt_sb[:, 2:4, :], in_=xt[p, :, 2:4, :])
            w12_sb = w_pool.tile([128, KT1 + KT2, 256], FP8, tag="w",
                                 name="w12_sb")
            nc.sync.dma_start(out=w12_sb[:], in_=w12[p])

        h1_sb = h1_pool.tile([128, MT, B], FP8, tag="h1", name="h1_sb")
        h2_sb = h2_pool.tile([128, KT2, B], FP8, tag="h2", name="h2_sb")

        # GEMM1: psum[o, b] = sum_i (16*W1)[i, o] * xT[i, b], fp8 DoubleRow.
        # Half-batch PSUM groups (2 banks) keep 4 groups in flight; whole
        # groups alternate between the Scalar and Vector eviction paths.
        # scalar Lrelu is 1 op, the vector mul+max pair is ~2x the time, so
        # scalar takes 5 of each pathway's 8 GEMM evict-groups; no two
        # vector groups are adjacent (vector bursts stall the PE on PSUM).
        sched = (1, 0, 1, 1, 0, 1, 1, 0)
        evi = [0]
        for m in range(MT):
            ph = [psg.tile([128, 2, 512], F32, tag="g", name="ps")
                  for _ in range(2)]
            for kp in range(KP1):
                for h in range(2):
                    for n in range(2):
                        _mm(nc, ph[h][:, n],
                            w12_sb[:, 2 * kp:2 * kp + 2,
                                   m * 128:(m + 1) * 128],
                            xt_sb[:, 2 * kp:2 * kp + 2,
                                  (2 * h + n) * 512:(2 * h + n + 1) * 512],
                            ldw=(h == 0 and n == 0),
                            start=(kp == 0),
                            stop=(kp == KP1 - 1),
                            perf_mode=DR)
            for h in range(2):
                _evict(nc, sc_pool, ph[h],
                       h1_sb[:, m, 2 * h * 512:(2 * h + 2) * 512],
                       use_scalar=sched[evi[0]])
                evi[0] += 1

        # GEMM2: one k-pair (K=256); the stationary is shared by all 4 chunks
        for m in range(MT):
            ph = [psg.tile([128, 2, 512], F32, tag="g", name="ps")
                  for _ in range(2)]
            for h in range(2):
                for n in range(2):
                    _mm(nc, ph[h][:, n],
                        w12_sb[:, KT1:KT1 + 2, m * 128:(m + 1) * 128],
                        h1_sb[:, 0:2,
                              (2 * h + n) * 512:(2 * h + n + 1) * 512],
                        ldw=(h == 0 and n == 0),
                        start=True,
                        stop=True,
                        perf_mode=DR)
            for h in range(2):
                _evict(nc, sc_pool, ph[h],
                       h2_sb[:, m, 2 * h * 512:(2 * h + 2) * 512],
                       use_scalar=sched[evi[0]])
                evi[0] += 1
        h2_tiles.append(h2_sb)

        # GEMV for a group of 4 pathways.  p is kept at 16x scale (p16 =
        # lrelu(16*h2w3)); BatchNorm absorbs the factor exactly once EPS is
        # scaled by 16^2, so no eviction rescale is needed anywhere.
        if p % 4 == 3 and GEMV_MODE == "dr4":
            # DoubleRow, zero-padded 4-column stationaries: pathway j's
            # weights live in column j, other columns are zero, and the 4
            # matmuls accumulate into the same [4, 512] PSUM chunk (+0 rows).
            # Both chunk-pair PSUM tiles stay live so each pathway's
            # stationary is loaded once.
            g = p // 4
            vt = sc_pool.tile([4, B], BF16, tag="vt", name="vt")
            pvs = [psg.tile([128, 2, 512], F32, tag="g", name="pv")
                   for _ in range(2)]
            for j in range(4):
                for nh in range(2):
                    for n in range(2):
                        _mm(nc, pvs[nh][0:4, n, :],
                            w3_sb[:, :, g * 4 + j, :],
                            h2_tiles[g * 4 + j][
                                :, :, (2 * nh + n) * 512:
                                (2 * nh + n + 1) * 512],
                            ldw=(nh == 0 and n == 0),
                            start=(j == 0),
                            stop=(j == 3),
                            perf_mode=DR)
            for nh in range(2):
                dstv = vt[:, nh * 1024:(nh + 1) * 1024]
                _evict(nc, sc_pool, pvs[nh][0:4, :, :], dstv,
                       use_scalar=(nh == 0))
                nc.sync.dma_start(
                    out=p_bf[:, g, nh * 1024:(nh + 1) * 1024],
                    in_=dstv)
            _bn_group(g)
        elif p % 4 == 3:
            # tile_position column packing, plain fp8 (baseline-proven).
            g = p // 4
            vt = sc_pool.tile([128, 4, 512], BF16, tag="vt", name="vt")
            for nh in range(2):
                pv = psg.tile([128, 2, 512], F32, tag="g", name="pv")
                for j in range(4):
                    for k in range(KT2):
                        for n in range(2):
                            _mm(nc, pv[32 * j:32 * j + 32, n, :],
                                w3tp_sb[:, g, k, 32 * j:32 * j + 32],
                                h2_tiles[g * 4 + j][
                                    :, k, (2 * nh + n) * 512:
                                    (2 * nh + n + 1) * 512],
                                ldw=(n == 0),
                                start=(k == 0),
                                stop=(k == KT2 - 1),
                                tile_position=(0, 32 * j))
                dstv = vt.rearrange("p a b -> p (a b)")[
                    :, nh * 1024:(nh + 1) * 1024]
                _evict(nc, sc_pool, pv, dstv, use_scalar=(nh == 0))
                nc.sync.dma_start(
                    out=p_bf[:, g, nh * 1024:(nh + 1) * 1024],
                    in_=vt[0:97:32, :, :].rearrange("p a b -> p (a b)")[
                        :, nh * 1024:(nh + 1) * 1024])
            _bn_group(g)

    # cross-partition reduce of [ssq_j, fcw_j*b_j] via a tiny fp32 matmul
    red = pss.tile([128, 2, 512], F32, tag="s", name="red")
    for g in range(2):
        nc.tensor.matmul(red[0:1, 0, 0:2], ones4[:], sfb[:, g, :],
                         start=(g == 0), stop=(g == 1))
    nc.scalar.activation(ssfb[:], red[0:1, 0, 0:2], AF.Copy)

    # one AllReduce: [s_row(2048), ssq, bias]
    ar_in = dram.tile([1, B + 2], F32)
    ar_out = dram.tile([1, B + 2], F32)
    nc.sync.dma_start(out=ar_in[0:1, 0:B], in_=s_row[:])
    nc.sync.dma_start(out=ar_in[0:1, B:B + 2], in_=ssfb[:])
    # dummy sigmoid depends on the last s_row eviction: it runs after every
    # other scalar op, loading the sigmoid table during the AllReduce wait.
    nc.scalar.activation(nt[:, 3:4], s_row[0:1, B - 1:B], AF.Sigmoid)
    nc.gpsimd.collective_compute(
        "AllReduce",
        ALU.add,
        replica_groups=[list(range(NCORES))],
        ins=[ar_in.opt()],
        outs=[ar_out.opt()],
    )
    nc.sync.dma_start(out=s128[:],
                      in_=ar_out[0:1, 0:B].rearrange("one (p j) -> p (one j)",
                                                     p=128))
    nc.sync.dma_start(out=tt2[:], in_=ar_out[0:1, B:B + 2])

    # rn = rsqrt(ssq_tot) via DVE: seed from reciprocal, 2 Newton steps
    y = nt[:, 0:1]
    t = nt[:, 1:2]
    rb = nt[:, 2:3]
    nc.vector.reciprocal(y[:], tt2[:, 0:1])
    nc.vector.tensor_scalar_mul(y[:], y[:], 1.0 / SEED_RN)  # y0 = S/c
    for _ in range(1):
        nc.vector.tensor_tensor(t[:], y[:], y[:], ALU.mult)
        nc.vector.tensor_tensor(t[:], t[:], tt2[:, 0:1], ALU.mult)
        nc.vector.tensor_scalar(t[:], t[:], -0.5, 1.5, ALU.mult, ALU.add)
        nc.vector.tensor_tensor(y[:], y[:], t[:], ALU.mult)
    nc.vector.tensor_tensor(rb[:], y[:], tt2[:, 1:2], ALU.mult)  # rn*bias
    nc.vector.tensor_scalar_mul(rnrb_bf[:, 0:1], y[:], 1.0)
    nc.vector.tensor_scalar_mul(rnrb_bf[:, 1:2], rb[:], 1.0)
    # broadcast [rn, rn*bias] to all 128 partitions via the PE
    bc = pss.tile([128, 2, 512], F32, tag="s", name="bc")
    nc.tensor.matmul(bc[0:128, 0, 0:2], ones_bf[:], rnrb_bf[:],
                     start=True, stop=True)
    nc.scalar.activation(rnb[:], bc[:, 0, 0:2], AF.Copy)
    # out = sigmoid(s*rn + cov + fc_b + rn*bias)
    nc.vector.scalar_tensor_tensor(u128[:], s128[:], rnb[:, 0:1], cov_row[:],
                                   ALU.mult, ALU.add)
    nc.scalar.activation(out_t[:], u128[:], AF.Sigmoid, bias=rnb[:, 1:2])
    nc.sync.dma_start(out=out.rearrange("(p j) one -> p (j one)", p=128),
                      in_=out_t[:])


def _dedup_ldweights(nc):
    """Drop InstLdweights that reload the stationary already in the PE.

    The Tile lowering splits every matmul into Ldweights + Matmult; for the
    n-inner loops above, consecutive groups load the identical stationary 4x.
    Only wait-free exact duplicates (same AP/offset/dtype/perf_mode/tile) with
    no intervening PE weight change are removed, so semaphore deps survive on
    the group's first load.
    """
    removed = 0
    for fn in nc.m.functions:
        for b in fn.blocks:
            last_sig = None
            keep = []
            for i in b.instructions:
                cn = type(i).__name__
                if cn == "InstLdweights":
                    ap = i.ins[0]
                    sig = (str(ap.ap), ap.offset, str(ap.dtype),
                           getattr(ap, "memref", None), str(i.perf_mode),
                           str(i.tile_position), str(i.tile_size),
                           str(i.is_transpose))
                    has_wait = bool(i.sync_info and i.sync_info.on_wait)
                    if sig == last_sig and not has_wait:
                        removed += 1
                        continue
                    last_sig = sig
                keep.append(i)
            b.instructions[:] = keep
    return removed


_NC = None


def _get_compiled():
    global _NC
    if _NC is None:
        nc = bacc.Bacc("TRN2", target_bir_lowering=False, debug=False,
                       num_devices=NCORES)
        xt = nc.dram_tensor("xt", [PPC, 128, KT1, B], FP8,
                            kind="ExternalInput").ap()
        w12 = nc.dram_tensor("w12", [PPC, 128, KT1 + KT2, 256], FP8,
                             kind="ExternalInput").ap()
        w3z = nc.dram_tensor("w3z", [128, KT2, PPC, 4], FP8,
                             kind="ExternalInput").ap()
        w3tp = nc.dram_tensor("w3tp", [128, 2, KT2, 128], FP8,
                              kind="ExternalInput").ap()
        xcovw = nc.dram_tensor("xcovw", [128, COV, 16], BF16,
                               kind="ExternalInput").ap()
        fcwcb = nc.dram_tensor("fcwcb", [128, COV], F32,
                               kind="ExternalInput").ap()
        fcbb = nc.dram_tensor("fcbb", [128, 1], F32,
                              kind="ExternalInput").ap()
        cst = nc.dram_tensor("cst", [4, 2, 5], F32,
                             kind="ExternalInput").ap()
        out = nc.dram_tensor("out", [B, 1], F32, kind="ExternalOutput").ap()
        with tile.TileContext(nc) as tc:
            with ExitStack() as ctx:
                _emit(ctx, tc, xt, w12, w3z, w3tp, xcovw, fcwcb, fcbb, cst,
                      out)
        _dedup_ldweights(nc)
        nc.compile()
        _NC = nc
    return _NC


def _shard(inputs):
    x = np.asarray(inputs["x"], np.float32)
    W1 = np.asarray(inputs["W1"], np.float32)
    W2 = np.asarray(inputs["W2"], np.float32)
    W3 = np.asarray(inputs["W3"], np.float32)
    gamma = np.asarray(inputs["gamma"], np.float32)
    beta = np.asarray(inputs["beta"], np.float32)
    fc_w = np.asarray(inputs["fc_w"], np.float32)
    fc_b = np.asarray(inputs["fc_b"], np.float32)

    fp8 = ml_dtypes.float8_e4m3
    xm = x[:, :P_TOT * NV].reshape(B, P_TOT, NV)
    # covariates laid out [partition(=b//16), c, j(=b%16)] for the DVE pass
    xcov = x[:, P_TOT * NV:P_TOT * NV + COV]          # [B, COV]
    xcovw = np.ascontiguousarray(
        xcov.reshape(128, 16, COV).transpose(0, 2, 1)).astype(
            ml_dtypes.bfloat16)
    fcwcb = np.broadcast_to(fc_w[P_TOT:P_TOT + COV].reshape(1, COV),
                            (128, COV)).astype(np.float32).copy()
    fcbb = np.full((128, 1), float(fc_b[0]), np.float32)

    maps = []
    for c in range(NCORES):
        sl = slice(c * PPC, (c + 1) * PPC)
        # xt: [PPC, 128, KT1, B]; feature f = kt*128 + kp
        xt_c = np.ascontiguousarray(
            xm[:, sl, :].transpose(1, 2, 0)            # [PPC, NV, B]
            .reshape(PPC, KT1, 128, B).transpose(0, 2, 1, 3)).astype(fp8)
        # w12: [PPC, 128, 6, 256] = [W1 k-tiles | W2 k-tiles]; only W1 is
        # prescaled -- the 16x rides through h1/h2/p and BN absorbs it.
        w1_c = (W1[sl] * WSCALE).reshape(PPC, KT1, 128, WID).transpose(
            0, 2, 1, 3)
        w2_c = W2[sl].reshape(PPC, KT2, 128, WID).transpose(
            0, 2, 1, 3)
        w12_c = np.concatenate([w1_c, w2_c], axis=2)
        w12_c = np.ascontiguousarray(w12_c).astype(fp8)
        # w3z: [128, KT2, PPC, 4]; pathway q's weights in column q%4
        w3z_c = np.zeros((128, KT2, PPC, 4), np.float32)
        # w3tp: [128, 2, KT2, 128]; pathway g*4+j in column 32*j
        w3tp_c = np.zeros((128, 2, KT2, 128), np.float32)
        for g in range(2):
            for j in range(4):
                wj = W3[c * PPC + g * 4 + j].reshape(KT2, 128)
                w3z_c[:, :, g * 4 + j, j] = wj.T
                w3tp_c[:, g, :, 32 * j] = wj.T
        w3z_c = w3z_c.astype(fp8)
        w3tp_c = w3tp_c.astype(fp8)
        gam = gamma[sl].astype(np.float32)
        bet = beta[sl].astype(np.float32)
        cst_c = np.stack([
            fc_w[sl, 0].astype(np.float32),
            gam,
            bet,
            B * gam * gam,
            B * bet * bet,
        ], axis=1).astype(np.float32)
        # [pathway, 5] -> [j, group, 5] with pathway = group*4 + j
        cst_c = np.ascontiguousarray(
            cst_c.reshape(2, 4, 5).transpose(1, 0, 2))
        maps.append({
            "xt": xt_c,
            "w12": w12_c,
            "w3z": w3z_c,
            "w3tp": w3tp_c,
            "xcovw": xcovw,
            "fcwcb": fcwcb,
            "fcbb": fcbb,
            "cst": np.ascontiguousarray(cst_c),
        })
    return maps


def kernel(**inputs) -> np.ndarray:
    nc = _get_compiled()
    maps = _shard(inputs)
    res = run_bass_kernel_spmd(nc, maps, list(range(NCORES)))
    return np.asarray(res.results[0]["out"], np.float32)


def kernel_traced(**inputs):
    """Like kernel() but with NTFF profiling; returns (out, BassKernelResults)."""
    nc = _get_compiled()
    maps = _shard(inputs)
    res = run_bass_kernel_spmd(nc, maps, list(range(NCORES)), trace=True)
    return np.asarray(res.results[0]["out"], np.float32), res


# revision 59
# speedup vs baseline: 1.0410x; 1.0277x over previous
"""DeepHisCoM forward pass on 8 Trainium2 NeuronCores.

Strategy: pathway (expert) parallelism -- 8 of the 64 pathways per core.
All three grouped GEMMs run in fp8e4 DoubleRow mode (2 k-rows per PE
cell per cycle); weights are host-prescaled by 16 to clear the fp8
subnormal floor and un-scaled for free inside the LeakyReLU eviction
(lrelu is positive-homogeneous: lrelu(u/16) = lrelu(u)/16).

PSUM evictions are split between the Scalar engine (native Lrelu) and
the Vector engine (mul+max pair) so the PE never stalls on a single
eviction engine and stays at its max p-state clock.

BatchNorm is per-pathway and therefore core-local.  The tail avoids
materializing pn entirely: with a = gamma*rstd and b = beta - mean*a,
  s_row[b]  = sum_j fcw_j*(a_j p_jb + b_j) = (fcw*a)^T p + sum_j fcw_j b_j
  ssq_j     = sum_b pn^2 = B*(a_j^2 var_j + beta_j^2)
so one tiny matmul over p (bf16, partitions 0-7) plus a handful of
[8,1] vector ops produce the AllReduce payload [s_row(2048), ssq, bias].
The s_row partials are DMAed PSUM->DRAM directly.

Post-AllReduce the final math runs on a [128,16] layout (16 batch
elements per partition) instead of a single partition; 1/||pn|| is
computed with a DVE Newton iteration (seeded by the near-constant
B*P ~ 131072 sum of squares) so the Scalar engine needs only the
sigmoid table, which is preloaded by a dummy op during the AllReduce
wait.
"""

import os
import sys

sys.path.insert(0, "/opt/trn_rl_repo")

from contextlib import ExitStack

import ml_dtypes
import numpy as np

import concourse.bacc as bacc
import concourse.bass as bass
import concourse.tile as tile
from concourse import mybir
from concourse.bass_utils import run_bass_kernel_spmd

P_TOT = 64   # pathways
NV = 512     # features per pathway
WID = 256    # hidden width
COV = 16     # covariates
B = 2048     # batch
EPS = 1e-5
SLOPE = 0.2
NCORES = 8
PPC = P_TOT // NCORES  # pathways per core
KT1 = NV // 128        # k-tiles for GEMM1 (4)
KT2 = WID // 128       # k-tiles for GEMM2 / GEMV (2)
KP1 = KT1 // 2         # k-pairs for GEMM1 DoubleRow (2)
MT = WID // 128        # m-tiles (2)
NCH = B // 512         # batch chunks of 512 (4)
# Host premultiplies W1 by 16 (clears the fp8 subnormal floor); W2/W3 stay
# unscaled, so h1/h2 carry the 16x factor through the fp8 pipeline (lrelu is
# positive-homogeneous) and only the GEMV eviction divides it back out.
WSCALE = 16.0
RSC = 1.0 / WSCALE
SEED_RN = 1.0 / 362.03867  # rsqrt seed: ||pn||^2 ~ B*P_TOT = 131072

FP8 = mybir.dt.float8e4
BF16 = mybir.dt.bfloat16
F32 = mybir.dt.float32
AF = mybir.ActivationFunctionType
ALU = mybir.AluOpType
DR = mybir.MatmulPerfMode.DoubleRow

# Native Lrelu runs on hardware; the CPU interpreter doesn't implement it,
# so sim checks set KERNEL_LRELU=0 to use the max(x, 0.2x) fallback.
USE_NATIVE_LRELU = os.environ.get("KERNEL_LRELU", "1") == "1"
# Vector-engine lrelu in one scalar_tensor_tensor (both sources PSUM);
# set KERNEL_VEC1=0 for the two-op mul+max fallback.
VEC1 = os.environ.get("KERNEL_VEC1", "1") == "1"
# GEMV mode: "dr4" = DoubleRow 4-column zero-padded accumulate,
# "tp" = tile_position column packing (plain fp8, baseline-proven).
GEMV_MODE = os.environ.get("KERNEL_GEMV", "dr4")


def _mm(nc, out, lhsT, rhs, ldw=True, **kw):
    """matmul wrapper: ldw=False reuses the previously loaded stationary."""
    mi = nc.tensor.matmul(out, lhsT, rhs, **kw)
    if not ldw:
        mi.ins.ldweights = False
    return mi


def _evict(nc, sc_pool, ps, dst, use_scalar):
    """dst = lrelu(ps), unscaled; ps is a PSUM [128, 2, 512] group.

    Whole groups alternate between the Scalar engine (native Lrelu, one op)
    and a GpSimd+Vector pair (0.2x to scratch on GpSimd, max on Vector) --
    each op reads PSUM only once, which is all the ISA allows.
    """
    src = ps.rearrange("p a b -> p (a b)")
    pp = src.shape[0]
    if use_scalar:
        if USE_NATIVE_LRELU:
            nc.scalar.activation(dst, src, AF.Lrelu, alpha=SLOPE)
            return
        sc = sc_pool.tile([128, 2 * 512], BF16, tag="scf", name="scf")
        nc.scalar.activation(sc[0:pp, :], src, AF.Copy, scale=SLOPE)
        nc.vector.tensor_tensor(dst, src, sc[0:pp, :], ALU.max)
        return
    sc = sc_pool.tile([128, 2 * 512], BF16, tag="scv", name="scv")
    nc.vector.tensor_scalar_mul(sc[0:pp, :], src, SLOPE)
    nc.vector.tensor_tensor(dst, src, sc[0:pp, :], ALU.max)


def _emit(ctx, tc, xt, w12, w3z, w3tp, xcovw, fcwcb, fcbb, cst, out):
    nc = tc.nc

    xt_pool = ctx.enter_context(tc.tile_pool(name="xt_pool", bufs=2))
    w_pool = ctx.enter_context(tc.tile_pool(name="w_pool", bufs=2))
    h1_pool = ctx.enter_context(tc.tile_pool(name="h1_pool", bufs=2))
    h2_pool = ctx.enter_context(tc.tile_pool(name="h2_pool", bufs=5))
    sc_pool = ctx.enter_context(tc.tile_pool(name="sc_pool", bufs=3))
    one = ctx.enter_context(tc.tile_pool(name="one", bufs=1))
    psg = ctx.enter_context(tc.tile_pool(name="psg", bufs=4, space="PSUM"))
    dram = ctx.enter_context(tc.tile_pool(name="dram", bufs=1, space="DRAM"))

    # ---- pathway 0's data first: it gates the first matmul ----
    xt_sb0 = xt_pool.tile([128, KT1, B], FP8, tag="xt", name="xt_sb")
    nc.sync.dma_start(out=xt_sb0[:, 0:2, :], in_=xt[0, :, 0:2, :])
    w12_sb0 = w_pool.tile([128, KT1 + KT2, 256], FP8, tag="w", name="w12_sb")
    nc.sync.dma_start(out=w12_sb0[:], in_=w12[0])
    nc.sync.dma_start(out=xt_sb0[:, 2:4, :], in_=xt[0, :, 2:4, :])

    # ---- persistents (all small; only needed mid-kernel or later) ----
    if GEMV_MODE == "dr4":
        w3_sb = one.tile([128, KT2, PPC, 4], FP8)
        nc.sync.dma_start(out=w3_sb[:], in_=w3z[:])
        w3tp_sb = None
    else:
        w3tp_sb = one.tile([128, 2, KT2, 128], FP8)
        nc.sync.dma_start(out=w3tp_sb[:], in_=w3tp[:])
        w3_sb = None
    # [j, group, (fcw, gamma, beta, B*g^2, B*b^2)] for pathway g*4+j
    cst_sb = one.tile([4, 2, 5], F32)
    nc.sync.dma_start(out=cst_sb[:], in_=cst[:])
    xcovw_sb = one.tile([128, COV, 16], BF16)
    nc.sync.dma_start(out=xcovw_sb[:], in_=xcovw[:])
    fcwcb_sb = one.tile([128, COV], F32)
    nc.sync.dma_start(out=fcwcb_sb[:], in_=fcwcb[:])
    fcbb_sb = one.tile([128, 1], F32)
    nc.sync.dma_start(out=fcbb_sb[:], in_=fcbb[:])
    ones_bf = one.tile([1, 128], BF16)
    nc.vector.memset(ones_bf[:], 1.0)

    # per-group BN state: groups of 4 pathways live on partitions 0-3 with
    # the group index in the free dim, so group 0's whole chain runs
    # mid-kernel while pathways 4-7 are still in their GEMMs.
    p_bf = one.tile([4, 2, B], BF16)
    stats = one.tile([4, 2, NCH, 6], F32)
    mv = one.tile([4, 2, 2], F32)
    ve = one.tile([4, 2], F32)
    rve = one.tile([4, 2], F32)
    rstd = one.tile([4, 2], F32)
    a_sc = one.tile([4, 2], F32)
    fcwa_f = one.tile([4, 2], F32)
    fcwa_bf = one.tile([4, 2], BF16)
    mb = one.tile([4, 2], F32)
    bvec = one.tile([4, 2], F32)
    vr = one.tile([4, 2], F32)
    sfb = one.tile([4, 2, 2], F32)        # [., group, (ssq_j | fcw_j*b_j)]
    ones4 = one.tile([4, 1], F32)
    s_g0 = one.tile([1, B], F32)
    s_row = one.tile([1, B], F32)
    ssfb = one.tile([1, 2], F32)         # [ssq partial, fcw*b partial]
    cov_row = one.tile([128, 16], F32)    # covariate term + fc_b, b = p*16+j
    s128 = one.tile([128, 16], F32)
    tt2 = one.tile([1, 2], F32)          # [ssq total, bias total]
    nt = one.tile([1, 4], F32)            # Newton scratch: y, t, rb, dummy
    rnrb_bf = one.tile([1, 2], BF16)
    rnb = one.tile([128, 2], F32)
    u128 = one.tile([128, 16], F32)
    out_t = one.tile([128, 16], F32)

    nc.vector.memset(ones4[:], 1.0)

    # ---- covariate term on the Vector engine (head is DMA-bound) ----
    # cov_row = sum_c fcwc_c * xcovw[:, c, :] + fc_b
    nc.vector.tensor_scalar(cov_row[:], xcovw_sb[:, 0, :],
                            fcwcb_sb[:, 0:1], fcbb_sb[:],
                            ALU.mult, ALU.add)
    for c in range(1, COV):
        nc.vector.scalar_tensor_tensor(cov_row[:], xcovw_sb[:, c, :],
                                       fcwcb_sb[:, c:c + 1], cov_row[:],
                                       ALU.mult, ALU.add)

    def _bn_group(g):
        """BN stats + analytic coefficients + s-partial matmuls for one
        group of 4 pathways.  Group 0 runs mid-kernel (hidden under the
        pathways 4-7 GEMMs); only group 1 is on the tail critical path."""
        gs = slice(g, g + 1)
        for s in range(NCH):
            nc.vector.bn_stats(out=stats[:, g, s, :],
                               in_=p_bf[:, g, s * 512:(s + 1) * 512])
        nc.vector.bn_aggr(out=mv[:, g, :], in_=stats[:, g])
        # p is carried at 16x scale, so eps scales by 16^2 (exact algebra)
        nc.vector.tensor_scalar_add(ve[:, gs], mv[:, g, 1:2],
                                    EPS * WSCALE * WSCALE)
        nc.vector.reciprocal(rve[:, gs], ve[:, gs])
        nc.scalar.activation(rstd[:, gs], rve[:, gs], AF.Sqrt)
        nc.vector.tensor_tensor(a_sc[:, gs], cst_sb[:, g, 1:2], rstd[:, gs],
                                ALU.mult)
        nc.vector.tensor_tensor(fcwa_f[:, gs], cst_sb[:, g, 0:1],
                                a_sc[:, gs], ALU.mult)
        nc.scalar.activation(fcwa_bf[:, gs], fcwa_f[:, gs], AF.Copy)
        # b = beta - mean*a; sfb[., 1] = fcw*b
        nc.vector.tensor_tensor(mb[:, gs], mv[:, g, 0:1], a_sc[:, gs],
                                ALU.mult)
        nc.vector.tensor_tensor(bvec[:, gs], cst_sb[:, g, 2:3], mb[:, gs],
                                ALU.subtract)
        nc.vector.tensor_tensor(sfb[:, g, 1:2], cst_sb[:, g, 0:1],
                                bvec[:, gs], ALU.mult)
        # sfb[., 0] = ssq_j = B*gamma^2 * var/(var+eps) + B*beta^2
        nc.vector.tensor_tensor(vr[:, gs], mv[:, g, 1:2], rve[:, gs],
                                ALU.mult)
        nc.vector.scalar_tensor_tensor(sfb[:, g, 0:1], vr[:, gs],
                                       cst_sb[:, g, 3:4], cst_sb[:, g, 4:5],
                                       ALU.mult, ALU.add)

    def _s_group(g):
        """s-partial matmuls for one group; runs after the pathway loop so
        the GEMM PSUM pool is free.  Group 1's eviction folds in group 0."""
        for sh in range(2):
            sp = psg.tile([128, 2, 512], F32, tag="g", name="sp")
            for n in range(2):
                _mm(nc, sp[0:1, n, :], fcwa_bf[:, g:g + 1],
                    p_bf[:, g, (2 * sh + n) * 512:(2 * sh + n + 1) * 512],
                    ldw=(n == 0), start=True, stop=True)
            srcs = sp[0:1, :, :].rearrange("p a b -> p (a b)")
            if g == 0:
                dsts = s_g0[:, sh * 1024:(sh + 1) * 1024]
                if sh == 0:
                    nc.scalar.activation(dsts, srcs, AF.Copy)
                else:
                    nc.vector.tensor_scalar_mul(dsts, srcs, 1.0)
            else:
                # fold in group 0's partial during the eviction
                nc.vector.scalar_tensor_tensor(
                    s_row[:, sh * 1024:(sh + 1) * 1024], srcs, 1.0,
                    s_g0[:, sh * 1024:(sh + 1) * 1024], ALU.mult, ALU.add)

    # ---- pathway loop ----
    h2_tiles = []
    for p in range(PPC):
        if p == 0:
            xt_sb, w12_sb = xt_sb0, w12_sb0
        else:
            xt_sb = xt_pool.tile([128, KT1, B], FP8, tag="xt", name="xt_sb")
            nc.sync.dma_start(out=xt_sb[:, 0:2, :], in_=xt[p, :, 0:2, :])
            nc.sync.dma_start(out=xt_sb[:, 2:4, :], in_=xt[p, :, 2:4, :])
            w12_sb = w_pool.tile([128, KT1 + KT2, 256], FP8, tag="w",
                                 name="w12_sb")
            nc.sync.dma_start(out=w12_sb[:], in_=w12[p])

        h1_sb = h1_pool.tile([128, MT, B], FP8, tag="h1", name="h1_sb")
        h2_sb = h2_pool.tile([128, KT2, B], FP8, tag="h2", name="h2_sb")

        # GEMM1: psum[o, b] = sum_i (16*W1)[i, o] * xT[i, b], fp8 DoubleRow.
        # Half-batch PSUM groups (2 banks) keep 4 groups in flight; whole
        # groups alternate between the Scalar and Vector eviction paths.
        # scalar Lrelu is 1 op, the vector mul+max pair is ~2x the time, so
        # scalar takes 5 of each pathway's 8 GEMM evict-groups; no two
        # vector groups are adjacent (vector bursts stall the PE on PSUM).
        sched = (1, 0, 1, 1, 0, 1, 1, 0)
        evi = [0]
        for m in range(MT):
            ph = [psg.tile([128, 2, 512], F32, tag="g", name="ps")
                  for _ in range(2)]
            for kp in range(KP1):
                for h in range(2):
                    for n in range(2):
                        _mm(nc, ph[h][:, n],
                            w12_sb[:, 2 * kp:2 * kp + 2,
                                   m * 128:(m + 1) * 128],
                            xt_sb[:, 2 * kp:2 * kp + 2,
                                  (2 * h + n) * 512:(2 * h + n + 1) * 512],
                            ldw=(h == 0 and n == 0),
                            start=(kp == 0),
                            stop=(kp == KP1 - 1),
                            perf_mode=DR)
            for h in range(2):
                _evict(nc, sc_pool, ph[h],
                       h1_sb[:, m, 2 * h * 512:(2 * h + 2) * 512],
                       use_scalar=sched[evi[0]])
                evi[0] += 1

        # GEMM2: one k-pair (K=256); the stationary is shared by all 4 chunks
        for m in range(MT):
            ph = [psg.tile([128, 2, 512], F32, tag="g", name="ps")
                  for _ in range(2)]
            for h in range(2):
                for n in range(2):
                    _mm(nc, ph[h][:, n],
                        w12_sb[:, KT1:KT1 + 2, m * 128:(m + 1) * 128],
                        h1_sb[:, 0:2,
                              (2 * h + n) * 512:(2 * h + n + 1) * 512],
                        ldw=(h == 0 and n == 0),
                        start=True,
                        stop=True,
                        perf_mode=DR)
            for h in range(2):
                _evict(nc, sc_pool, ph[h],
                       h2_sb[:, m, 2 * h * 512:(2 * h + 2) * 512],
                       use_scalar=sched[evi[0]])
                evi[0] += 1
        h2_tiles.append(h2_sb)

        # GEMV for a group of 4 pathways.  p is kept at 16x scale (p16 =
        # lrelu(16*h2w3)); BatchNorm absorbs the factor exactly once EPS is
        # scaled by 16^2, so no eviction rescale is needed anywhere.
        if p % 4 == 3 and GEMV_MODE == "dr4":
            # DoubleRow, zero-padded 4-column stationaries: pathway j's
            # weights live in column j, other columns are zero, and the 4
            # matmuls accumulate into the same [4, 512] PSUM chunk (+0 rows).
            # Both chunk-pair PSUM tiles stay live so each pathway's
            # stationary is loaded once.
            g = p // 4
            vt = sc_pool.tile([4, B], BF16, tag="vt", name="vt")
            pvs = [psg.tile([128, 2, 512], F32, tag="g", name="pv")
                   for _ in range(2)]
            for j in range(4):
                for nh in range(2):
                    for n in range(2):
                        _mm(nc, pvs[nh][0:4, n, :],
                            w3_sb[:, :, g * 4 + j, :],
                            h2_tiles[g * 4 + j][
                                :, :, (2 * nh + n) * 512:
                                (2 * nh + n + 1) * 512],
                            ldw=(nh == 0 and n == 0),
                            start=(j == 0),
                            stop=(j == 3),
                            perf_mode=DR)
            for nh in range(2):
                dstv = vt[:, nh * 1024:(nh + 1) * 1024]
                # scalar takes the last pair: it gates the tail stats
                _evict(nc, sc_pool, pvs[nh][0:4, :, :], dstv,
                       use_scalar=(nh == 1))
                nc.sync.dma_start(
                    out=p_bf[:, g, nh * 1024:(nh + 1) * 1024],
                    in_=dstv)
            _bn_group(g)
        elif p % 4 == 3:
            # tile_position column packing, plain fp8 (baseline-proven).
            g = p // 4
            vt = sc_pool.tile([128, 4, 512], BF16, tag="vt", name="vt")
            for nh in range(2):
                pv = psg.tile([128, 2, 512], F32, tag="g", name="pv")
                for j in range(4):
                    for k in range(KT2):
                        for n in range(2):
                            _mm(nc, pv[32 * j:32 * j + 32, n, :],
                                w3tp_sb[:, g, k, 32 * j:32 * j + 32],
                                h2_tiles[g * 4 + j][
                                    :, k, (2 * nh + n) * 512:
                                    (2 * nh + n + 1) * 512],
                                ldw=(n == 0),
                                start=(k == 0),
                                stop=(k == KT2 - 1),
                                tile_position=(0, 32 * j))
                dstv = vt.rearrange("p a b -> p (a b)")[
                    :, nh * 1024:(nh + 1) * 1024]
                _evict(nc, sc_pool, pv, dstv, use_scalar=(nh == 0))
                nc.sync.dma_start(
                    out=p_bf[:, g, nh * 1024:(nh + 1) * 1024],
                    in_=vt[0:97:32, :, :].rearrange("p a b -> p (a b)")[
                        :, nh * 1024:(nh + 1) * 1024])
            _bn_group(g)

    _s_group(0)
    _s_group(1)
    # cross-partition reduce of [ssq_j, fcw_j*b_j] via a tiny fp32 matmul
    red = psg.tile([128, 2, 512], F32, tag="g", name="red")
    for g in range(2):
        nc.tensor.matmul(red[0:1, 0, 0:2], ones4[:], sfb[:, g, :],
                         start=(g == 0), stop=(g == 1))
    nc.scalar.activation(ssfb[:], red[0:1, 0, 0:2], AF.Copy)

    # one AllReduce: [s_row(2048), ssq, bias]
    ar_in = dram.tile([1, B + 2], F32)
    ar_out = dram.tile([1, B + 2], F32)
    nc.sync.dma_start(out=ar_in[0:1, 0:B], in_=s_row[:])
    nc.sync.dma_start(out=ar_in[0:1, B:B + 2], in_=ssfb[:])
    # dummy sigmoid depends on the last s_row eviction: it runs after every
    # other scalar op, loading the sigmoid table during the AllReduce wait.
    nc.scalar.activation(nt[:, 3:4], s_row[0:1, B - 1:B], AF.Sigmoid)
    nc.gpsimd.collective_compute(
        "AllReduce",
        ALU.add,
        replica_groups=[list(range(NCORES))],
        ins=[ar_in.opt()],
        outs=[ar_out.opt()],
    )
    nc.sync.dma_start(out=s128[:],
                      in_=ar_out[0:1, 0:B].rearrange("one (p j) -> p (one j)",
                                                     p=128))
    nc.sync.dma_start(out=tt2[:], in_=ar_out[0:1, B:B + 2])

    # rn = rsqrt(ssq_tot) via DVE: seed from reciprocal, 2 Newton steps
    y = nt[:, 0:1]
    t = nt[:, 1:2]
    rb = nt[:, 2:3]
    nc.vector.reciprocal(y[:], tt2[:, 0:1])
    nc.vector.tensor_scalar_mul(y[:], y[:], 1.0 / SEED_RN)  # y0 = S/c
    for _ in range(1):
        nc.vector.tensor_tensor(t[:], y[:], y[:], ALU.mult)
        nc.vector.tensor_tensor(t[:], t[:], tt2[:, 0:1], ALU.mult)
        nc.vector.tensor_scalar(t[:], t[:], -0.5, 1.5, ALU.mult, ALU.add)
        nc.vector.tensor_tensor(y[:], y[:], t[:], ALU.mult)
    nc.vector.tensor_tensor(rb[:], y[:], tt2[:, 1:2], ALU.mult)  # rn*bias
    nc.vector.tensor_scalar_mul(rnrb_bf[:, 0:1], y[:], 1.0)
    nc.vector.tensor_scalar_mul(rnrb_bf[:, 1:2], rb[:], 1.0)
    # broadcast [rn, rn*bias] to all 128 partitions via the PE
    bc = psg.tile([128, 2, 512], F32, tag="g", name="bc")
    nc.tensor.matmul(bc[0:128, 0, 0:2], ones_bf[:], rnrb_bf[:],
                     start=True, stop=True)
    nc.scalar.activation(rnb[:], bc[:, 0, 0:2], AF.Copy)
    # out = sigmoid(s*rn + cov + fc_b + rn*bias)
    nc.vector.scalar_tensor_tensor(u128[:], s128[:], rnb[:, 0:1], cov_row[:],
                                   ALU.mult, ALU.add)
    nc.scalar.activation(out_t[:], u128[:], AF.Sigmoid, bias=rnb[:, 1:2])
    nc.sync.dma_start(out=out.rearrange("(p j) one -> p (j one)", p=128),
                      in_=out_t[:])


def _dedup_ldweights(nc):
    """Drop InstLdweights that reload the stationary already in the PE.

    The Tile lowering splits every matmul into Ldweights + Matmult; for the
    n-inner loops above, consecutive groups load the identical stationary 4x.
    Only wait-free exact duplicates (same AP/offset/dtype/perf_mode/tile) with
    no intervening PE weight change are removed, so semaphore deps survive on
    the group's first load.
    """
    removed = 0
    for fn in nc.m.functions:
        for b in fn.blocks:
            last_sig = None
            keep = []
            for i in b.instructions:
                cn = type(i).__name__
                if cn == "InstLdweights":
                    ap = i.ins[0]
                    sig = (str(ap.ap), ap.offset, str(ap.dtype),
                           getattr(ap, "memref", None), str(i.perf_mode),
                           str(i.tile_position), str(i.tile_size),
                           str(i.is_transpose))
                    has_wait = bool(i.sync_info and i.sync_info.on_wait)
                    if sig == last_sig and not has_wait:
                        removed += 1
                        continue
                    last_sig = sig
                keep.append(i)
            b.instructions[:] = keep
    return removed


_NC = None


def _get_compiled():
    global _NC
    if _NC is None:
        nc = bacc.Bacc("TRN2", target_bir_lowering=False, debug=False,
                       num_devices=NCORES)
        xt = nc.dram_tensor("xt", [PPC, 128, KT1, B], FP8,
                            kind="ExternalInput").ap()
        w12 = nc.dram_tensor("w12", [PPC, 128, KT1 + KT2, 256], FP8,
                             kind="ExternalInput").ap()
        w3z = nc.dram_tensor("w3z", [128, KT2, PPC, 4], FP8,
                             kind="ExternalInput").ap()
        w3tp = nc.dram_tensor("w3tp", [128, 2, KT2, 128], FP8,
                              kind="ExternalInput").ap()
        xcovw = nc.dram_tensor("xcovw", [128, COV, 16], BF16,
                               kind="ExternalInput").ap()
        fcwcb = nc.dram_tensor("fcwcb", [128, COV], F32,
                               kind="ExternalInput").ap()
        fcbb = nc.dram_tensor("fcbb", [128, 1], F32,
                              kind="ExternalInput").ap()
        cst = nc.dram_tensor("cst", [4, 2, 5], F32,
                             kind="ExternalInput").ap()
        out = nc.dram_tensor("out", [B, 1], F32, kind="ExternalOutput").ap()
        with tile.TileContext(nc) as tc:
            with ExitStack() as ctx:
                _emit(ctx, tc, xt, w12, w3z, w3tp, xcovw, fcwcb, fcbb, cst,
                      out)
        _dedup_ldweights(nc)
        nc.compile()
        _NC = nc
    return _NC


def _shard(inputs):
    x = np.asarray(inputs["x"], np.float32)
    W1 = np.asarray(inputs["W1"], np.float32)
    W2 = np.asarray(inputs["W2"], np.float32)
    W3 = np.asarray(inputs["W3"], np.float32)
    gamma = np.asarray(inputs["gamma"], np.float32)
    beta = np.asarray(inputs["beta"], np.float32)
    fc_w = np.asarray(inputs["fc_w"], np.float32)
    fc_b = np.asarray(inputs["fc_b"], np.float32)

    fp8 = ml_dtypes.float8_e4m3
    xm = x[:, :P_TOT * NV].reshape(B, P_TOT, NV)
    # covariates laid out [partition(=b//16), c, j(=b%16)] for the DVE pass
    xcov = x[:, P_TOT * NV:P_TOT * NV + COV]          # [B, COV]
    xcovw = np.ascontiguousarray(
        xcov.reshape(128, 16, COV).transpose(0, 2, 1)).astype(
            ml_dtypes.bfloat16)
    fcwcb = np.broadcast_to(fc_w[P_TOT:P_TOT + COV].reshape(1, COV),
                            (128, COV)).astype(np.float32).copy()
    fcbb = np.full((128, 1), float(fc_b[0]), np.float32)

    maps = []
    for c in range(NCORES):
        sl = slice(c * PPC, (c + 1) * PPC)
        # xt: [PPC, 128, KT1, B]; feature f = kt*128 + kp
        xt_c = np.ascontiguousarray(
            xm[:, sl, :].transpose(1, 2, 0)            # [PPC, NV, B]
            .reshape(PPC, KT1, 128, B).transpose(0, 2, 1, 3)).astype(fp8)
        # w12: [PPC, 128, 6, 256] = [W1 k-tiles | W2 k-tiles]; only W1 is
        # prescaled -- the 16x rides through h1/h2/p and BN absorbs it.
        w1_c = (W1[sl] * WSCALE).reshape(PPC, KT1, 128, WID).transpose(
            0, 2, 1, 3)
        w2_c = W2[sl].reshape(PPC, KT2, 128, WID).transpose(
            0, 2, 1, 3)
        w12_c = np.concatenate([w1_c, w2_c], axis=2)
        w12_c = np.ascontiguousarray(w12_c).astype(fp8)
        # w3z: [128, KT2, PPC, 4]; pathway q's weights in column q%4
        w3z_c = np.zeros((128, KT2, PPC, 4), np.float32)
        # w3tp: [128, 2, KT2, 128]; pathway g*4+j in column 32*j
        w3tp_c = np.zeros((128, 2, KT2, 128), np.float32)
        for g in range(2):
            for j in range(4):
                wj = W3[c * PPC + g * 4 + j].reshape(KT2, 128)
                w3z_c[:, :, g * 4 + j, j] = wj.T
                w3tp_c[:, g, :, 32 * j] = wj.T
        w3z_c = w3z_c.astype(fp8)
        w3tp_c = w3tp_c.astype(fp8)
        gam = gamma[sl].astype(np.float32)
        bet = beta[sl].astype(np.float32)
        cst_c = np.stack([
            fc_w[sl, 0].astype(np.float32),
            gam,
            bet,
            B * gam * gam,
            B * bet * bet,
        ], axis=1).astype(np.float32)
        # [pathway, 5] -> [j, group, 5] with pathway = group*4 + j
        cst_c = np.ascontiguousarray(
            cst_c.reshape(2, 4, 5).transpose(1, 0, 2))
        maps.append({
            "xt": xt_c,
            "w12": w12_c,
            "w3z": w3z_c,
            "w3tp": w3tp_c,
            "xcovw": xcovw,
            "fcwcb": fcwcb,
            "fcbb": fcbb,
            "cst": np.ascontiguousarray(cst_c),
        })
    return maps


def kernel(**inputs) -> np.ndarray:
    nc = _get_compiled()
    maps = _shard(inputs)
    res = run_bass_kernel_spmd(nc, maps, list(range(NCORES)))
    return np.asarray(res.results[0]["out"], np.float32)


def kernel_traced(**inputs):
    """Like kernel() but with NTFF profiling; returns (out, BassKernelResults)."""
    nc = _get_compiled()
    maps = _shard(inputs)
    res = run_bass_kernel_spmd(nc, maps, list(range(NCORES)), trace=True)
    return np.asarray(res.results[0]["out"], np.float32), res
